# revision 41
# baseline (speedup 1.0000x reference)
"""Single-head attention (shared-input QKV projections) on 8 Trainium2 cores.

Reference computation (per batch b):
    q = x[b] @ Wq; k = x[b] @ Wk; v = x[b] @ Wv        # [S, 64]
    out[b] = softmax(q @ k.T / 8) @ v                  # [S, 64]
with B=4, S=4096, D=256, OUT=64.

Sharding: data-parallel over batch (4 batches x 2 cores) with
sequence-parallel query halves. All 8 cores run one SPMD program; the
per-core query offset is handled by host-side row rotation of x[b]
(attention is permutation-invariant over key/value rows), so core c gets
x rotated by (c%2)*2048 rows and computes attention for its first 2048
rows against all 4096 keys.

Host-side staging (free): x is passed pre-transposed (d-major x^T) so the
device needs no transposes or layout copies for the projections, and the
projection weights are passed duplicated along the output dim so Q^T/K^T
land duplicated across both 64-partition halves, enabling 2-way PE
row-packing of the K=64 score matmuls.

Per-core kernel (all matmuls float32r = TF32-class, ~1e-4 rel err):
  1. DMA x^T and W straight into float32r SBUF.
  2. Projections Q^T/K^T (duplicated) and V^T; V^T is PE-transposed into
     natural V chunks with an appended ones column (so attn @ V_aug also
     yields the softmax denominator for free).
  3. Scores computed transposed (S^T[k, q]) so no attention transpose is
     needed: per (q-block, k-chunk-pair): 2 row-packed K=64 matmuls ->
     PSUM [128, 2W]; one ACT exp (scale=1/8, max-subtraction skipped --
     scores are bounded ~|4|) -> SBUF; 2 accumulating attn @ V_aug
     matmuls into PSUM [65, W].
  4. Epilogue: PE transpose [65,128]->[128,65], reciprocal of the ones
     row, per-partition scale, DMA out.
K-chunk production (projections, V assembly) is emitted interleaved with
q-block 0's consumption so the ACT exp stream starts within a few us.
"""

import numpy as np

import concourse.mybir as mybir
import concourse.tile as tile
from concourse import bacc
from concourse.masks import make_identity

P = 128
D = 256
OUT = 64
SCALE = 0.125
F32 = mybir.dt.float32
F32R = mybir.dt.float32r
BF16 = mybir.dt.bfloat16

B_FULL, S_FULL = 4, 4096
N_CORES = 8
JUNK_WARM = 0


def build_nc(S: int, QH: int, QB_W: int = 512, loop_n: int | None = None,
             timing_mode: bool = False):
    """Build the per-core SPMD program.

    S: sequence length (key/value rows) held by this core.
    QH: number of query rows this core computes (first QH rows of x).
    QB_W: query block width (free dim of the score matmuls).
    loop_n: if set, run the whole body loop_n times on device (for timing).
    timing_mode: shrink the xt input to 512 cols (replicated on device) so
        host->device transfer noise doesn't swamp loop-delta timing.
    """
    assert S % 512 == 0 and QH % QB_W == 0 and QB_W % P == 0
    nc = bacc.Bacc()
    xt_cols = 512 if timing_mode else S
    xt_in = nc.declare_dram_parameter("xt", [2, P, xt_cols], F32R, isOutput=False)
    w_in = nc.declare_dram_parameter("w", [3, D, P], F32R, isOutput=False)
    out_d = nc.declare_dram_parameter("out", [QH, OUT], F32, isOutput=True)

    with tile.TileContext(nc) as tc:
        with (
            tc.tile_pool(name="const", bufs=1) as constp,
            tc.tile_pool(name="big", bufs=1) as bigp,
            tc.tile_pool(name="attnp", bufs=20) as attnp,
            tc.tile_pool(name="epil", bufs=2) as epilp,
            tc.tile_pool(name="outp", bufs=4) as outp,
            tc.tile_pool(name="miscps", bufs=2, space="PSUM") as miscps,
            tc.tile_pool(name="stps", bufs=2, space="PSUM") as stps,
            tc.tile_pool(name="pops", bufs=2, space="PSUM") as pops,
        ):
            ident = constp.tile([P, P], F32)
            make_identity(nc, ident)
            # Weights split across the two HWDGE queues (SP carries q/k,
            # ACT carries v) so the first projection's deps land early.
            w_sb = constp.tile([P, 6 * P], F32R)
            for j in range(3):
                eng = nc.sync if j < 2 else nc.scalar
                for c in range(2):
                    eng.dma_start(
                        w_sb[:, (j * 2 + c) * P : (j * 2 + c + 1) * P],
                        w_in[j, c * P : (c + 1) * P, :],
                    )
            # Shared (kt/vt) and per-half double-buffered (xt/qt/v_sb)
            # tensors. The loop body is 2x unrolled: half B's input DMAs and
            # head projections overlap half A's tail, so the only expensive
            # seam is the once-per-two-iterations For_i barrier. v_sb's ones
            # columns are written once (the per-chunk V copies never touch
            # column 64), so init them outside the loop.
            nk = S // P
            kt = bigp.tile([P, S], F32R)
            vt = bigp.tile([P, S], F32)
            ones32 = constp.tile([P, nk], F32)
            nc.vector.memset(ones32, 1.0)
            n_half = 2 if loop_n is not None else 1
            xts, qts, v_sbs = [], [], []
            for h in range(n_half):
                xts.append(bigp.tile([P, 2 * S], F32R, name=f"xt{h}"))
                qts.append(bigp.tile([P, QH], F32R, name=f"qt{h}"))
                v_sb = bigp.tile([P, nk * 65], BF16, name=f"v_sb{h}")
                nc.vector.tensor_copy(
                    v_sb.rearrange("p (k c) -> p k c", c=65)[:, :, 64], ones32
                )
                v_sbs.append(v_sb)
            shared = (nc, tc, xt_in, out_d, S, QH, QB_W, constp, bigp,
                      attnp, epilp, outp, miscps, stps, pops, ident, w_sb,
                      kt, vt, timing_mode)
            if loop_n is not None:
                assert loop_n % 2 == 0, "loop_n must be even (2x unroll)"
                loop_cm = tc.For_i(0, loop_n // 2, 1)
                loop_cm.__enter__()
                tail = _emit_body(*shared, xts[0], qts[0], v_sbs[0],
                                  pending_tail=None, defer_tail=True)
                _emit_body(*shared, xts[1], qts[1], v_sbs[1],
                           pending_tail=tail, defer_tail=False)
                loop_cm.__exit__(None, None, None)
            else:
                _emit_body(*shared, xts[0], qts[0], v_sbs[0],
                           pending_tail=None, defer_tail=False)
    return nc


def _emit_body(nc, tc, xt_in, out_d, S, QH, QB_W, constp, bigp, attnp,
               epilp, outp, miscps, stps, pops, ident, w_sb, kt, vt,
               timing_mode, xt, qt, v_sb, pending_tail=None,
               defer_tail=False):
    nk = S // P          # 128-row k chunks
    npair = nk // 2      # row-packed chunk pairs
    nqb = QH // QB_W     # q blocks
    qpb = min(512, QH)   # q-projection block width

    # x^T: chunk c at cols [c*S, (c+1)*S). Two small leading slices let the
    # first projections start early; the rest transfers as one big DMA per
    # chunk (fewer HWDGE queue slots — each dma_start occupies the queue
    # ~0.6us regardless of size). Chunk 0 rides the SP HWDGE queue, chunk 1
    # the ACT queue, so the two halves of any column range land in parallel.
    if timing_mode:
        # identical DMA volume to the real build, but from a small hot HBM
        # region (same protocol the baseline measurement used)
        slices = [(lo, 512) for lo in range(0, S, 512)]
    else:
        slices = [(0, 512), (512, 512)]
        if S > 1024:
            slices.append((1024, S - 1024))
    for lo, wdt in slices:
        for c in range(2):
            eng = nc.sync if c == 0 else nc.scalar
            src_lo = 0 if timing_mode else lo
            eng.dma_start(
                xt[:, c * S + lo : c * S + lo + wdt],
                xt_in[c, :, src_lo : src_lo + wdt],
            )

    # Previous unroll-half's tail (last exp+AV) lands here: its PE work
    # overlaps this half's input DMA latency. Its epilogue is deferred to
    # this half's head_done point so the epilogue's serial DVE chain and
    # PSUM pool rotation don't sit between this half's first score matmuls.
    tail_epi = None
    if pending_tail is not None:
        tail_epi = pending_tail()
    else:
        # Trip start: PE sat idle through the loop barrier (> the ~3.4us HAM
        # window) and would run the head chain at the throttled clock. Burn
        # a few dependency-free matmuls on resident weights during the xt
        # DMA wait to re-warm it.
        for _ in range(JUNK_WARM):
            junk = miscps.tile([P, 4 * P], F32, name="junk", tag="mps")
            nc.tensor.matmul(junk, w_sb[:, 0:P], w_sb[:, 0:4 * P],
                             start=True, stop=True)

    def proj_block(dst, j, lo, width):
        """dst[:, lo:lo+width] = (W_j^T x^T)[:, lo:lo+width] (d contracted)."""
        pp = miscps.tile([P, width], F32, name="pp", tag="mps")
        for c in range(2):
            nc.tensor.matmul(
                pp,
                w_sb[:, (j * 2 + c) * P : (j * 2 + c + 1) * P],
                xt[:, c * S + lo : c * S + lo + width],
                start=(c == 0),
                stop=(c == 1),
            )
        nc.vector.tensor_copy(dst[:, lo : lo + width], pp)

    def v_chunk(kc):
        """v_sb chunk kc = V rows [kc*128,(kc+1)*128) via PE transpose."""
        tv = miscps.tile([P, OUT], F32, name="tv", tag="mps")
        nc.tensor.transpose(
            tv, vt[0:64, kc * P : (kc + 1) * P], ident[0:64, 0:64]
        )
        nc.vector.tensor_copy(v_sb[:, kc * 65 : kc * 65 + 64], tv)

    po_tiles = {}

    def st_part(qb, t):
        """Score matmuls (S^T) for q block qb, k chunk pair t -> PSUM tile."""
        qs = qb * QB_W
        kca, kcb = 2 * t, 2 * t + 1
        st = stps.tile([P, 2 * QB_W], F32, name="st", tag="st")
        nc.tensor.matmul(
            st[:, 0:QB_W],
            kt[0:64, kca * P : (kca + 1) * P],
            qt[0:64, qs : qs + QB_W],
            start=True,
            stop=True,
        )
        nc.tensor.matmul(
            st[:, QB_W : 2 * QB_W],
            kt[64:128, kcb * P : (kcb + 1) * P],
            qt[64:128, qs : qs + QB_W],
            start=True,
            stop=True,
        )
        return st

    def exp_part(st):
        """exp of a score tile -> SBUF attn tile (bf16, like v_sb)."""
        at = attnp.tile([P, 2 * QB_W], BF16, name="at", tag="at")
        nc.scalar.activation(
            at, st, mybir.ActivationFunctionType.Exp, scale=SCALE
        )
        return at

    def av_part(qb, t, at):
        """attn@V_aug accumulation for q block qb, k chunk pair t."""
        kca, kcb = 2 * t, 2 * t + 1
        po = po_tiles[qb]
        nc.tensor.matmul(
            po,
            v_sb[:, kca * 65 : (kca + 1) * 65],
            at[:, 0:QB_W],
            start=(t == 0),
            stop=False,
        )
        nc.tensor.matmul(
            po,
            v_sb[:, kcb * 65 : (kcb + 1) * 65],
            at[:, QB_W : 2 * QB_W],
            start=False,
            stop=(t == npair - 1),
        )

    def exp_av_part(qb, t, st):
        av_part(qb, t, exp_part(st))

    def main_pair(qb, t):
        exp_av_part(qb, t, st_part(qb, t))

    def epilogue(qb):
        qs = qb * QB_W
        po = po_tiles.pop(qb)
        o_sb = epilp.tile([65, QB_W], F32, name="o_sb", tag="o_sb")
        nc.vector.tensor_copy(o_sb, po)
        # One [128, 4*64] result tile and a single out-DMA per q block: each
        # dma_start occupies its HWDGE queue ~0.6us regardless of size, so 4
        # separate row-block DMAs would serialize into the kernel tail.
        ob = outp.tile([P, (QB_W // P) * OUT], F32, name="ob", tag="ob")
        for jj in range(QB_W // P):
            tr = miscps.tile([P, 65], F32, name="tr", tag="mps")
            nc.tensor.transpose(
                tr, o_sb[:, jj * P : (jj + 1) * P], ident[0:65, 0:65]
            )
            rs = outp.tile([P, 1], F32, name="rs", tag="rs")
            nc.vector.reciprocal(rs, tr[:, 64:65])
            nc.vector.tensor_scalar_mul(
                ob[:, jj * OUT : (jj + 1) * OUT], tr[:, 0:64], rs
            )
        nc.sync.dma_start(
            out_d[qs : qs + QB_W, :].rearrange("(j p) o -> p j o", p=P),
            ob.rearrange("p (j o) -> p j o", o=OUT),
        )

    # --- emission: interleave k-chunk production with the first q blocks
    # (phase 1 feeds ACT from ~two q blocks while PE also runs the
    # projections; remaining q blocks are pure ACT-bound streaming) ---
    lead = min(2, nqb)
    defer_qb = lead if nqb > lead else None  # 3rd q block: exp in phase 1, AV deferred
    n_qt = lead + (1 if defer_qb is not None else 0)
    qsplit = min(-(-(n_qt * QB_W) // qpb) * qpb, QH)
    # Only q block 0 is projected before the k/v pipeline starts; the other
    # lead blocks are deferred until after the first score matmul so the
    # first exp fires as early as possible.
    proj_block(qt, 0, 0, min(qpb, qsplit))
    for qb in range(lead):
        po_tiles[qb] = pops.tile([65, QB_W], F32, name="po", tag="po")
    # Phase 1, software-pipelined one combo deep across the whole phase:
    # each combo's score matmuls are emitted before the previous combo's
    # exp/attn@V so the exp stream never waits on a fresh S^T + semaphore.
    deferred = {}
    pend = None

    def flush_pend():
        nonlocal pend
        if pend is None:
            return
        qb_p, t_p, st_p = pend
        if qb_p == defer_qb:
            deferred[t_p] = exp_part(st_p)
        else:
            exp_av_part(qb_p, t_p, st_p)
        pend = None

    head_done = False
    for g in range(S // 512):  # 512 k rows per group = 4 chunks = 2 pairs
        proj_block(kt, 1, g * 512, 512)
        if head_done:
            proj_block(vt, 2, g * 512, 512)
            for kc in range(4 * g, 4 * g + 4):
                v_chunk(kc)
        for t in (2 * g, 2 * g + 1):
            combos = list(range(lead)) + ([defer_qb] if defer_qb is not None else [])
            for qb in combos:
                st = st_part(qb, t)
                if not head_done:
                    # First score matmul is in flight: now emit everything
                    # that was held back to shorten the path to the first exp
                    # (remaining lead q projections, V for group 0, and the
                    # previous unroll half's epilogue).
                    for lo in range(qpb, qsplit, qpb):
                        proj_block(qt, 0, lo, min(qpb, QH - lo))
                    proj_block(vt, 2, 0, 512)
                    for kc in range(4):
                        v_chunk(kc)
                    if tail_epi is not None:
                        tail_epi()
                        tail_epi = None
                    head_done = True
                flush_pend()
                pend = (qb, t, st)
    flush_pend()
    if tail_epi is not None:  # safety: head_done insert never fired
        tail_epi()
        tail_epi = None
    for lo in range(qsplit, QH, qpb):
        proj_block(qt, 0, lo, min(qpb, QH - lo))
    for qb in range(lead):
        epilogue(qb)
    if defer_qb is not None:
        po_tiles[defer_qb] = pops.tile([65, QB_W], F32, name="po", tag="po")
    # Phase 2: pure streaming q blocks, software-pipelined one pair deep so
    # the next pair's score matmuls are already queued on the PE while ACT
    # runs the current exp (closes the per-pair sem-latency gap on ACT).
    # Remaining q blocks, software-pipelined one pair deep; the deferred
    # q block's attn@V matmuls (PE-only) are interleaved with the next
    # block's pairs so ACT never idles at the phase boundary.
    rest = list(range(lead + (1 if defer_qb is not None else 0), nqb))
    tail_fn = None
    for qb in rest:
        po_tiles[qb] = pops.tile([65, QB_W], F32, name="po", tag="po")
        pend = None
        for t in range(npair):
            st = st_part(qb, t)
            if deferred and qb == rest[0]:
                av_part(defer_qb, t, deferred.pop(t))
                if not deferred:
                    # deferred q block is complete: drain it now so its
                    # epilogue + out-DMA overlap the last block's exp stream
                    epilogue(defer_qb)
            if pend is not None:
                exp_av_part(qb, pend[0], pend[1])
            pend = (t, st)
        if qb == rest[-1] and defer_tail:
            # Hand the very last exp+AV (and, one step later, the epilogue)
            # to the next unroll half: emitted after its input DMAs, so this
            # tail hides their latency.
            pend_t, pend_st = pend

            def tail_fn(qb=qb, pend_t=pend_t, pend_st=pend_st):
                exp_av_part(qb, pend_t, pend_st)
                return lambda: epilogue(qb)
        else:
            exp_av_part(qb, pend[0], pend[1])
            epilogue(qb)
    if defer_qb is not None and not rest:
        for t in range(npair):
            av_part(defer_qb, t, deferred.pop(t))
        epilogue(defer_qb)
    return tail_fn


_compiled_nc = None
LAST_RESULT = None  # BassKernelResults of the most recent kernel() call


def _get_compiled_nc():
    global _compiled_nc
    if _compiled_nc is None:
        nc = build_nc(S_FULL, S_FULL // 2)
        nc.compile()
        _compiled_nc = nc
    return _compiled_nc


def make_in_maps(x, w):
    """Host-side staging: roll per query half, transpose to d-major,
    duplicate weights along the output dim."""
    qh = S_FULL // 2
    wdup = np.ascontiguousarray(np.concatenate([w, w], axis=2))  # [3,256,128]
    in_maps = []
    for c in range(N_CORES):
        b, h = c // 2, c % 2
        xb = x[b]
        xr = xb if h == 0 else np.concatenate([xb[qh:], xb[:qh]], axis=0)
        xtc = np.ascontiguousarray(xr.T).reshape(2, P, S_FULL)
        in_maps.append({"xt": xtc, "w": wdup})
    return in_maps


def kernel(x, kernel):
    from concourse.bass_utils import run_bass_kernel_spmd

    x = np.asarray(x, dtype=np.float32)
    w = np.asarray(kernel, dtype=np.float32)
    assert x.shape == (B_FULL, S_FULL, D) and w.shape == (3, D, OUT)
    qh = S_FULL // 2

    nc = _get_compiled_nc()
    res = run_bass_kernel_spmd(nc, make_in_maps(x, w), core_ids=list(range(N_CORES)))
    global LAST_RESULT
    LAST_RESULT = res
    out = np.empty((B_FULL, S_FULL, OUT), dtype=np.float32)
    for c in range(N_CORES):
        b, h = c // 2, c % 2
        out[b, h * qh : (h + 1) * qh] = res.results[c]["out"]
    return out



# revision 42
# speedup vs baseline: 1.1451x; 1.1451x over previous
"""Single-head attention (shared-input QKV projections) on 8 Trainium2 cores.

Reference computation (per batch b):
    q = x[b] @ Wq; k = x[b] @ Wk; v = x[b] @ Wv        # [S, 64]
    out[b] = softmax(q @ k.T / 8) @ v                  # [S, 64]
with B=4, S=4096, D=256, OUT=64.

Sharding: data-parallel over batch (4 batches x 2 cores) with
sequence-parallel query halves. All 8 cores run one SPMD program; the
per-core query offset is handled by host-side row rotation of x[b]
(attention is permutation-invariant over key/value rows), so core c gets
x rotated by (c%2)*2048 rows and computes attention for its first 2048
rows against all 4096 keys.

Host-side staging (free): x is passed pre-transposed (d-major x^T) so the
device needs no transposes or layout copies for the projections, and the
projection weights are passed duplicated along the output dim so Q^T/K^T
land duplicated across both 64-partition halves, enabling 2-way PE
row-packing of the K=64 score matmuls.

Per-core kernel (all matmuls float32r = TF32-class, ~1e-4 rel err):
  1. DMA x^T and W straight into float32r SBUF.
  2. Projections Q^T/K^T (duplicated) and V^T; V^T is PE-transposed into
     natural V chunks with an appended ones column (so attn @ V_aug also
     yields the softmax denominator for free).
  3. Scores computed transposed (S^T[k, q]) so no attention transpose is
     needed: per (q-block, k-chunk-pair): 2 row-packed K=64 matmuls ->
     PSUM [128, 2W]; one ACT exp (scale=1/8, max-subtraction skipped --
     scores are bounded ~|4|) -> SBUF; 2 accumulating attn @ V_aug
     matmuls into PSUM [65, W].
  4. Epilogue: PE transpose [65,128]->[128,65], reciprocal of the ones
     row, per-partition scale, DMA out.
K-chunk production (projections, V assembly) is emitted interleaved with
q-block 0's consumption so the ACT exp stream starts within a few us.
"""

import numpy as np

import concourse.mybir as mybir
import concourse.tile as tile
from concourse import bacc
from concourse.masks import make_identity

P = 128
D = 256
OUT = 64
SCALE = 0.125
F32 = mybir.dt.float32
F32R = mybir.dt.float32r
BF16 = mybir.dt.bfloat16

B_FULL, S_FULL = 4, 4096
N_CORES = 8
JUNK_WARM = 0


def build_nc(S: int, QH: int, QB_W: int = 512, loop_n: int | None = None,
             timing_mode: bool = False):
    """Build the per-core SPMD program.

    S: sequence length (key/value rows) held by this core.
    QH: number of query rows this core computes (first QH rows of x).
    QB_W: query block width (free dim of the score matmuls).
    loop_n: if set, run the whole body loop_n times on device (for timing).
    timing_mode: shrink the xt input to 512 cols (replicated on device) so
        host->device transfer noise doesn't swamp loop-delta timing.
    """
    assert S % 512 == 0 and QH % QB_W == 0 and QB_W % P == 0
    nc = bacc.Bacc()
    xt_cols = 512 if timing_mode else S
    xt_in = nc.declare_dram_parameter("xt", [2, P, xt_cols], F32R, isOutput=False)
    w_in = nc.declare_dram_parameter("w", [3, D, P], F32R, isOutput=False)
    out_d = nc.declare_dram_parameter("out", [QH, OUT], F32, isOutput=True)

    with tile.TileContext(nc) as tc:
        with (
            tc.tile_pool(name="const", bufs=1) as constp,
            tc.tile_pool(name="big", bufs=1) as bigp,
            tc.tile_pool(name="attnp", bufs=20) as attnp,
            tc.tile_pool(name="epil", bufs=2) as epilp,
            tc.tile_pool(name="outp", bufs=4) as outp,
            tc.tile_pool(name="miscps", bufs=2, space="PSUM") as miscps,
            tc.tile_pool(name="stps", bufs=2, space="PSUM") as stps,
            tc.tile_pool(name="pops", bufs=2, space="PSUM") as pops,
        ):
            ident = constp.tile([P, P], F32)
            make_identity(nc, ident)
            # Weights split across the two HWDGE queues (SP carries q/k,
            # ACT carries v) so the first projection's deps land early.
            w_sb = constp.tile([P, 6 * P], F32R)
            for j in range(3):
                eng = nc.sync if j < 2 else nc.scalar
                for c in range(2):
                    eng.dma_start(
                        w_sb[:, (j * 2 + c) * P : (j * 2 + c + 1) * P],
                        w_in[j, c * P : (c + 1) * P, :],
                    )
            # Shared (kt/vt) and per-half double-buffered (xt/qt/v_sb)
            # tensors. The loop body is 2x unrolled: half B's input DMAs and
            # head projections overlap half A's tail, so the only expensive
            # seam is the once-per-two-iterations For_i barrier. v_sb's ones
            # columns are written once (the per-chunk V copies never touch
            # column 64), so init them outside the loop.
            nk = S // P
            kt = bigp.tile([P, S], F32R)
            vt = bigp.tile([P, S], F32)
            ones32 = constp.tile([P, nk], F32)
            nc.vector.memset(ones32, 1.0)
            n_half = 2 if loop_n is not None else 1
            xts, qts, v_sbs = [], [], []
            for h in range(n_half):
                xts.append(bigp.tile([P, 2 * S], F32R, name=f"xt{h}"))
                qts.append(bigp.tile([P, QH], F32R, name=f"qt{h}"))
                v_sb = bigp.tile([P, nk * 65], BF16, name=f"v_sb{h}")
                nc.vector.tensor_copy(
                    v_sb.rearrange("p (k c) -> p k c", c=65)[:, :, 64], ones32
                )
                v_sbs.append(v_sb)
            shared = (nc, tc, xt_in, out_d, S, QH, QB_W, constp, bigp,
                      attnp, epilp, outp, miscps, stps, pops, ident, w_sb,
                      kt, vt, timing_mode)
            if loop_n is not None:
                assert loop_n % 2 == 0, "loop_n must be even (2x unroll)"
                loop_cm = tc.For_i(0, loop_n // 2, 1)
                loop_cm.__enter__()
                tail = _emit_body(*shared, xts[0], qts[0], v_sbs[0],
                                  pending_tail=None, defer_tail=True)
                _emit_body(*shared, xts[1], qts[1], v_sbs[1],
                           pending_tail=tail, defer_tail=False)
                loop_cm.__exit__(None, None, None)
            else:
                _emit_body(*shared, xts[0], qts[0], v_sbs[0],
                           pending_tail=None, defer_tail=False)
    return nc


def _emit_body(nc, tc, xt_in, out_d, S, QH, QB_W, constp, bigp, attnp,
               epilp, outp, miscps, stps, pops, ident, w_sb, kt, vt,
               timing_mode, xt, qt, v_sb, pending_tail=None,
               defer_tail=False):
    nk = S // P          # 128-row k chunks
    npair = nk // 2      # row-packed chunk pairs
    nqb = QH // QB_W     # q blocks
    qpb = min(512, QH)   # q-projection block width

    # x^T: chunk c at cols [c*S, (c+1)*S). Two small leading slices let the
    # first projections start early; the rest transfers as one big DMA per
    # chunk (fewer HWDGE queue slots — each dma_start occupies the queue
    # ~0.6us regardless of size). Chunk 0 rides the SP HWDGE queue, chunk 1
    # the ACT queue, so the two halves of any column range land in parallel.
    if timing_mode:
        # identical DMA volume to the real build, but from a small hot HBM
        # region (same protocol the baseline measurement used)
        slices = [(lo, 512) for lo in range(0, S, 512)]
    else:
        slices = [(0, 512), (512, 512)]
        if S > 1024:
            slices.append((1024, S - 1024))
    # Chunk 1 rides the gpsimd SWDGE queue, NOT the ACT HWDGE queue: DMA
    # issue occupies the issuing engine's sequencer ~0.5us each, and a burst
    # of xt DMAs on the ACT queue at the unroll seam delays the first exp of
    # this half by several us. The gpsimd sequencer is otherwise idle.
    for lo, wdt in slices:
        for c in range(2):
            eng = nc.sync if c == 0 else nc.gpsimd
            src_lo = 0 if timing_mode else lo
            eng.dma_start(
                xt[:, c * S + lo : c * S + lo + wdt],
                xt_in[c, :, src_lo : src_lo + wdt],
            )

    # Previous unroll-half's tail (last exp+AV) lands here: its PE work
    # overlaps this half's input DMA latency. Its epilogue is deferred to
    # this half's head_done point so the epilogue's serial DVE chain and
    # PSUM pool rotation don't sit between this half's first score matmuls.
    tail_epi = None
    if pending_tail is not None:
        tail_epi = pending_tail()
    else:
        # Trip start: PE sat idle through the loop barrier (> the ~3.4us HAM
        # window) and would run the head chain at the throttled clock. Burn
        # a few dependency-free matmuls on resident weights during the xt
        # DMA wait to re-warm it.
        for _ in range(JUNK_WARM):
            junk = miscps.tile([P, 4 * P], F32, name="junk", tag="mps")
            nc.tensor.matmul(junk, w_sb[:, 0:P], w_sb[:, 0:4 * P],
                             start=True, stop=True)

    def proj_block(dst, j, lo, width):
        """dst[:, lo:lo+width] = (W_j^T x^T)[:, lo:lo+width] (d contracted)."""
        pp = miscps.tile([P, width], F32, name="pp", tag="mps")
        for c in range(2):
            nc.tensor.matmul(
                pp,
                w_sb[:, (j * 2 + c) * P : (j * 2 + c + 1) * P],
                xt[:, c * S + lo : c * S + lo + width],
                start=(c == 0),
                stop=(c == 1),
            )
        nc.vector.tensor_copy(dst[:, lo : lo + width], pp)

    def v_chunk(kc):
        """v_sb chunk kc = V rows [kc*128,(kc+1)*128) via PE transpose."""
        tv = miscps.tile([P, OUT], F32, name="tv", tag="mps")
        nc.tensor.transpose(
            tv, vt[0:64, kc * P : (kc + 1) * P], ident[0:64, 0:64]
        )
        nc.vector.tensor_copy(v_sb[:, kc * 65 : kc * 65 + 64], tv)

    po_tiles = {}

    def st_part(qb, t):
        """Score matmuls (S^T) for q block qb, k chunk pair t -> PSUM tile."""
        qs = qb * QB_W
        kca, kcb = 2 * t, 2 * t + 1
        st = stps.tile([P, 2 * QB_W], F32, name="st", tag="st")
        nc.tensor.matmul(
            st[:, 0:QB_W],
            kt[0:64, kca * P : (kca + 1) * P],
            qt[0:64, qs : qs + QB_W],
            start=True,
            stop=True,
        )
        nc.tensor.matmul(
            st[:, QB_W : 2 * QB_W],
            kt[64:128, kcb * P : (kcb + 1) * P],
            qt[64:128, qs : qs + QB_W],
            start=True,
            stop=True,
        )
        return st

    def exp_part(st):
        """exp of a score tile -> SBUF attn tile (bf16, like v_sb)."""
        at = attnp.tile([P, 2 * QB_W], BF16, name="at", tag="at")
        nc.scalar.activation(
            at, st, mybir.ActivationFunctionType.Exp, scale=SCALE
        )
        return at

    def av_part(qb, t, at):
        """attn@V_aug accumulation for q block qb, k chunk pair t."""
        kca, kcb = 2 * t, 2 * t + 1
        po = po_tiles[qb]
        nc.tensor.matmul(
            po,
            v_sb[:, kca * 65 : (kca + 1) * 65],
            at[:, 0:QB_W],
            start=(t == 0),
            stop=False,
        )
        nc.tensor.matmul(
            po,
            v_sb[:, kcb * 65 : (kcb + 1) * 65],
            at[:, QB_W : 2 * QB_W],
            start=False,
            stop=(t == npair - 1),
        )

    def exp_av_part(qb, t, st):
        av_part(qb, t, exp_part(st))

    def main_pair(qb, t):
        exp_av_part(qb, t, st_part(qb, t))

    def epilogue(qb):
        qs = qb * QB_W
        po = po_tiles.pop(qb)
        o_sb = epilp.tile([65, QB_W], F32, name="o_sb", tag="o_sb")
        nc.vector.tensor_copy(o_sb, po)
        # One [128, 4*64] result tile and a single out-DMA per q block: each
        # dma_start occupies its HWDGE queue ~0.6us regardless of size, so 4
        # separate row-block DMAs would serialize into the kernel tail.
        ob = outp.tile([P, (QB_W // P) * OUT], F32, name="ob", tag="ob")
        for jj in range(QB_W // P):
            tr = miscps.tile([P, 65], F32, name="tr", tag="mps")
            nc.tensor.transpose(
                tr, o_sb[:, jj * P : (jj + 1) * P], ident[0:65, 0:65]
            )
            rs = outp.tile([P, 1], F32, name="rs", tag="rs")
            nc.vector.reciprocal(rs, tr[:, 64:65])
            nc.vector.tensor_scalar_mul(
                ob[:, jj * OUT : (jj + 1) * OUT], tr[:, 0:64], rs
            )
        nc.sync.dma_start(
            out_d[qs : qs + QB_W, :].rearrange("(j p) o -> p j o", p=P),
            ob.rearrange("p (j o) -> p j o", o=OUT),
        )

    # --- emission: interleave k-chunk production with the first q blocks
    # (phase 1 feeds ACT from ~two q blocks while PE also runs the
    # projections; remaining q blocks are pure ACT-bound streaming) ---
    lead = min(2, nqb)
    defer_qb = lead if nqb > lead else None  # 3rd q block: exp in phase 1, AV deferred
    n_qt = lead + (1 if defer_qb is not None else 0)
    qsplit = min(-(-(n_qt * QB_W) // qpb) * qpb, QH)
    # Only q block 0 is projected before the k/v pipeline starts; the other
    # lead blocks are deferred until after the first score matmul so the
    # first exp fires as early as possible.
    proj_block(qt, 0, 0, min(qpb, qsplit))
    for qb in range(lead):
        po_tiles[qb] = pops.tile([65, QB_W], F32, name="po", tag="po")
    # Phase 1, software-pipelined one combo deep across the whole phase:
    # each combo's score matmuls are emitted before the previous combo's
    # exp/attn@V so the exp stream never waits on a fresh S^T + semaphore.
    deferred = {}
    pend = None

    def flush_pend():
        nonlocal pend
        if pend is None:
            return
        qb_p, t_p, st_p = pend
        if qb_p == defer_qb:
            deferred[t_p] = exp_part(st_p)
        else:
            exp_av_part(qb_p, t_p, st_p)
        pend = None

    head_done = False
    for g in range(S // 512):  # 512 k rows per group = 4 chunks = 2 pairs
        proj_block(kt, 1, g * 512, 512)
        if head_done:
            proj_block(vt, 2, g * 512, 512)
            for kc in range(4 * g, 4 * g + 4):
                v_chunk(kc)
        for t in (2 * g, 2 * g + 1):
            combos = list(range(lead)) + ([defer_qb] if defer_qb is not None else [])
            for qb in combos:
                st = st_part(qb, t)
                if not head_done:
                    # First score matmul is in flight: now emit everything
                    # that was held back to shorten the path to the first exp
                    # (remaining lead q projections, V for group 0, and the
                    # previous unroll half's epilogue).
                    for lo in range(qpb, qsplit, qpb):
                        proj_block(qt, 0, lo, min(qpb, QH - lo))
                    proj_block(vt, 2, 0, 512)
                    for kc in range(4):
                        v_chunk(kc)
                    if tail_epi is not None:
                        tail_epi()
                        tail_epi = None
                    head_done = True
                flush_pend()
                pend = (qb, t, st)
    flush_pend()
    if tail_epi is not None:  # safety: head_done insert never fired
        tail_epi()
        tail_epi = None
    for lo in range(qsplit, QH, qpb):
        proj_block(qt, 0, lo, min(qpb, QH - lo))
    for qb in range(lead):
        epilogue(qb)
    if defer_qb is not None:
        po_tiles[defer_qb] = pops.tile([65, QB_W], F32, name="po", tag="po")
    # Phase 2: pure streaming q blocks, software-pipelined one pair deep so
    # the next pair's score matmuls are already queued on the PE while ACT
    # runs the current exp (closes the per-pair sem-latency gap on ACT).
    # Remaining q blocks, software-pipelined one pair deep; the deferred
    # q block's attn@V matmuls (PE-only) are interleaved with the next
    # block's pairs so ACT never idles at the phase boundary.
    rest = list(range(lead + (1 if defer_qb is not None else 0), nqb))
    tail_fn = None
    for qb in rest:
        po_tiles[qb] = pops.tile([65, QB_W], F32, name="po", tag="po")
        pend = None
        for t in range(npair):
            st = st_part(qb, t)
            if deferred and qb == rest[0]:
                av_part(defer_qb, t, deferred.pop(t))
                if not deferred:
                    # deferred q block is complete: drain it now so its
                    # epilogue + out-DMA overlap the last block's exp stream
                    epilogue(defer_qb)
            if pend is not None:
                exp_av_part(qb, pend[0], pend[1])
            pend = (t, st)
        if qb == rest[-1] and defer_tail:
            # Hand the very last exp+AV (and, one step later, the epilogue)
            # to the next unroll half: emitted after its input DMAs, so this
            # tail hides their latency.
            pend_t, pend_st = pend

            def tail_fn(qb=qb, pend_t=pend_t, pend_st=pend_st):
                exp_av_part(qb, pend_t, pend_st)
                return lambda: epilogue(qb)
        else:
            exp_av_part(qb, pend[0], pend[1])
            epilogue(qb)
    if defer_qb is not None and not rest:
        for t in range(npair):
            av_part(defer_qb, t, deferred.pop(t))
        epilogue(defer_qb)
    return tail_fn


_compiled_nc = None
LAST_RESULT = None  # BassKernelResults of the most recent kernel() call


def _get_compiled_nc():
    global _compiled_nc
    if _compiled_nc is None:
        nc = build_nc(S_FULL, S_FULL // 2)
        nc.compile()
        _compiled_nc = nc
    return _compiled_nc


def make_in_maps(x, w):
    """Host-side staging: roll per query half, transpose to d-major,
    duplicate weights along the output dim."""
    qh = S_FULL // 2
    wdup = np.ascontiguousarray(np.concatenate([w, w], axis=2))  # [3,256,128]
    in_maps = []
    for c in range(N_CORES):
        b, h = c // 2, c % 2
        xb = x[b]
        xr = xb if h == 0 else np.concatenate([xb[qh:], xb[:qh]], axis=0)
        xtc = np.ascontiguousarray(xr.T).reshape(2, P, S_FULL)
        in_maps.append({"xt": xtc, "w": wdup})
    return in_maps


def kernel(x, kernel):
    from concourse.bass_utils import run_bass_kernel_spmd

    x = np.asarray(x, dtype=np.float32)
    w = np.asarray(kernel, dtype=np.float32)
    assert x.shape == (B_FULL, S_FULL, D) and w.shape == (3, D, OUT)
    qh = S_FULL // 2

    nc = _get_compiled_nc()
    res = run_bass_kernel_spmd(nc, make_in_maps(x, w), core_ids=list(range(N_CORES)))
    global LAST_RESULT
    LAST_RESULT = res
    out = np.empty((B_FULL, S_FULL, OUT), dtype=np.float32)
    for c in range(N_CORES):
        b, h = c // 2, c % 2
        out[b, h * qh : (h + 1) * qh] = res.results[c]["out"]
    return out

